# revision 1
# baseline (speedup 1.0000x reference)
"""AssociativeLayerWrapper kernel — self-contained.

Computes the fast-weight associate (read) pass over the full sequence plus the
delta-rule memory update on the last NUM_MEM_TOKENS tokens, matching the
reference bit-for-bit in float32.

Shapes (hardcoded per spec):
  hidden_states (4, 4096, 2048), W_mq (512, 2048), W_mk (512, 2048),
  W_mv (2048, 2048), W_mb_w (16, 2048), W_mb_b (16,),
  W_mem (4, 16, 192, 128), z (4, 16, 192)

Distribution: the computation is embarrassingly parallel over (batch, head)
— W_mem/z are per-sample state and every einsum is head-diagonal.  When the
8 axon-tunneled trn2 NeuronCores are reachable through jax we run the heavy
associate pass data-parallel over 8 shards (batch x sequence-half); otherwise
we fall back to a pure-numpy implementation.  The small update stage (128
tokens) runs locally either way.
"""

import numpy as np

NU = 3
N_HEADS = 16
NUM_MEM_TOKENS = 128
EPS_NORM = 1e-12
EPS_DENOM = 1e-5


def _dpfp_np(x):
    # x: (..., d) -> (..., 2*NU*d)
    y = np.concatenate([np.maximum(x, 0.0), np.maximum(-x, 0.0)], axis=-1)
    rolled = np.concatenate(
        [np.roll(y, j, axis=-1) for j in range(1, NU + 1)], axis=-1
    )
    rep = np.concatenate([y] * NU, axis=-1)
    return rep * rolled


def _l2n_np(x):
    n = np.linalg.norm(x, axis=-1, keepdims=True)
    return x / np.maximum(n, EPS_NORM)


def _to_heads(x):
    b, s, d = x.shape
    return x.reshape(b, s, N_HEADS, d // N_HEADS).transpose(0, 2, 1, 3)


def _from_heads(x):
    b, h, s, dh = x.shape
    return x.transpose(0, 2, 1, 3).reshape(b, s, h * dh)


def _associate_np(hidden, W_mq, W_mem, z):
    """read + residual for a token chunk.  hidden: (b, s, d_model)."""
    q = _to_heads(hidden @ W_mq.T)                      # (b,h,s,32)
    mq = _l2n_np(_dpfp_np(q))                           # (b,h,s,192)
    num = np.einsum("bhjk,bhkt->bhjt", mq, W_mem, optimize=True)
    denom = np.einsum("bhk,bhjk->bhj", z, mq, optimize=True)[..., None] + EPS_DENOM
    read = _from_heads(num / denom)                     # (b,s,d_model)
    return read + hidden


def _update_np(out, W_mq_unused, W_mk, W_mv, W_mb_w, W_mb_b, W_mem, z):
    mem = out[:, -NUM_MEM_TOKENS:]                      # (b,m,d_model)
    k = _to_heads(mem @ W_mk.T)
    mk = _l2n_np(_dpfp_np(k))                           # (b,h,m,192)
    new_mv = _to_heads(mem @ W_mv.T)                    # (b,h,m,128)
    num2 = np.einsum("bhjk,bhkt->bhjt", mk, W_mem, optimize=True)
    denom2 = np.einsum("bhj,bhkj->bhk", z, mk, optimize=True)[..., None] + EPS_DENOM
    prev_mv = num2 / denom2
    coef = np.clip(
        1.0 - denom2 / np.sum(mk * mk, axis=-1, keepdims=True), 0.0, 1.0
    )
    mv = new_mv - prev_mv
    mb = _to_heads(
        1.0 / (1.0 + np.exp(-(mem @ W_mb_w.T + W_mb_b)))
    )                                                   # (b,h,m,1)
    assoc = np.einsum(
        "bhjk,bhjt->bhkt", mk * mb, mv, optimize=True
    )
    W_mem_new = W_mem + assoc
    z_new = z + np.sum(coef * mk, axis=-2)
    return W_mem_new, z_new


def _kernel_numpy(hidden_states, W_mq, W_mk, W_mv, W_mb_w, W_mb_b, W_mem, z):
    out = _associate_np(hidden_states, W_mq, W_mem, z)
    W_mem_new, z_new = _update_np(out, W_mq, W_mk, W_mv, W_mb_w, W_mb_b, W_mem, z)
    return out, W_mem_new, z_new


def _try_kernel_jax(hidden_states, W_mq, W_mk, W_mv, W_mb_w, W_mb_b, W_mem, z):
    """Run the heavy associate pass sharded over the 8 trn2 NeuronCores.

    Shards: (batch b=4) x (sequence halves) = 8 ways.  Each shard carries its
    sample's W_mem/z (per-sample state, head-diagonal einsums).  Falls back by
    raising on any failure; caller catches.
    """
    import jax
    import jax.numpy as jnp

    devs = jax.devices()
    if len(devs) < 8:
        raise RuntimeError("need 8 devices")
    b, s, d_model = hidden_states.shape
    half = s // 2

    # Build 8 shards: shard i = (sample i//2, seq-half i%2)
    hs = hidden_states.reshape(b, 2, half, d_model).reshape(b * 2, half, d_model)
    Wm = np.repeat(W_mem, 2, axis=0)                    # (8,16,192,128)
    zz = np.repeat(z, 2, axis=0)                        # (8,16,192)

    def shard_fn(hidden, Wm_s, z_s):
        # hidden (half, d_model); Wm_s (h,192,128); z_s (h,192)
        q = (hidden @ W_mq_c.T).reshape(half, N_HEADS, -1).transpose(1, 0, 2)
        x = jnp.concatenate([jax.nn.relu(q), jax.nn.relu(-q)], axis=-1)
        rolled = jnp.concatenate(
            [jnp.roll(x, j, axis=-1) for j in range(1, NU + 1)], axis=-1
        )
        rep = jnp.concatenate([x] * NU, axis=-1)
        mq = rep * rolled
        n = jnp.linalg.norm(mq, axis=-1, keepdims=True)
        mq = mq / jnp.maximum(n, EPS_NORM)              # (h, half, 192)
        num = jnp.einsum("hjk,hkt->hjt", mq, Wm_s)
        denom = jnp.einsum("hk,hjk->hj", z_s, mq)[..., None] + EPS_DENOM
        read = (num / denom).transpose(1, 0, 2).reshape(half, d_model)
        return read + hidden

    W_mq_c = jnp.asarray(W_mq)
    out_shards = jax.pmap(shard_fn)(
        jnp.asarray(hs), jnp.asarray(Wm), jnp.asarray(zz)
    )
    out = np.asarray(out_shards).reshape(b, 2, half, d_model).reshape(b, s, d_model)
    out = out.astype(np.float32)
    W_mem_new, z_new = _update_np(out, W_mq, W_mk, W_mv, W_mb_w, W_mb_b, W_mem, z)
    return out, W_mem_new, z_new


def kernel(hidden_states, W_mq, W_mk, W_mv, W_mb_w, W_mb_b, W_mem, z):
    args = (hidden_states, W_mq, W_mk, W_mv, W_mb_w, W_mb_b, W_mem, z)
    args = tuple(np.asarray(a, dtype=np.float32) for a in args)
    try:
        return _try_kernel_jax(*args)
    except Exception:
        return _kernel_numpy(*args)
